# revision 6
# baseline (speedup 1.0000x reference)
"""Trainium2 Bass kernel for the SOCS lithography simulator.

Reference math (per batch b):
    aerial = sum_k s_k * | cIFFT2( cFFT2(mask_b) * pad_center(kernels[k]) ) |^2
    resist = sigmoid(50*(aerial - 0.225));  printed = (resist > 0.5)

The padded kernels live in the *frequency* domain with only a 35x35 window of
nonzero coefficients, so every field is band-limited to 35x35 frequencies and
aerial (a sum of |field|^2) is band-limited to 69x69.  Everything reduces to
small dense matmuls on a coarse grid of NC=70 >= 69 uniform samples per axis:

    Mhat  = A @ x @ A.T           A = rows 494:529 of the centered DFT matrix
    G_k   = Mhat * (sqrt(s_k) * kernels[k])                  [35,35] cplx
    W_k   = G_k @ C.T             C = inverse-DFT at y_m = 1024*m/70  [70,35]
    F_k   = C @ W_k               fields on the 70x70 coarse grid
    aer_c = sum_k |F_k|^2         exact coarse samples of aerial
    aerial = U @ aer_c @ U.T      U real [1024,70] Dirichlet interp (exact)

Complex products are folded into PE contractions via row-stacking:
  * stage 2c: lhsT = gt70b[99,70] per k (rows v / 64+v carry [GrT|GiT] and
    [-GiT|GrT]); rhs = cc99 [99,70] (rows ctr / cti) -> out [Wr;Wi] [70,70].
  * stage 2d: lhsT = [ctr;-cti] or [cti;ctr] [70,70]; rhs = W-stack with
    6 kernels batched in columns -> F_r/F_i [70,420] per group.

Precision: bf16 through stage 2, fp16 interpolation + fp16 outputs
(host upcasts to f32).  Simulated end-to-end rel l2 ~3.3e-3.

Sharding: 8 cores; core c handles batch c//2 and output row-half c%2.
No collectives.  Self-contained: shapes/constants hardcoded.
"""

import os

import numpy as np

N = 1024
B, K, HK = 4, 24, 35
PT = (N - HK) // 2          # 494
NC = 70                     # coarse grid (>= 2*HK-1 = 69)
NF = 2 * HK - 1             # 69 product frequencies
DOSE = 1.0
RESIST_THRESHOLD = 0.225
RESIST_STEEPNESS = 50.0
KG = 4                      # 2c/2d column-batched k-groups
KPG = K // KG               # kernels per group = 6


# ---------------------------------------------------------------- host matrices
def _host_matrices():
    """Input-independent constant matrices (f64 on host)."""
    u = np.arange(HK)[:, None]
    y = np.arange(N)[None, :]
    A = np.exp(-2j * np.pi * ((u + PT - N // 2) * (y - N // 2)) / N)  # [35,1024]
    atp = np.concatenate([A.real.T, A.imag.T], axis=1)                # [1024,70]

    m = np.arange(NC)
    ym = N * m / NC                                                   # fractional
    Ac = np.exp(-2j * np.pi * ((u + PT - N // 2) * (ym[None, :] - N // 2)) / N)
    C = np.conj(Ac).T / N                                             # [70,35]
    ctr = np.ascontiguousarray(C.real.T)                              # [35,70]
    cti = np.ascontiguousarray(C.imag.T)

    yy = np.arange(N)[:, None]
    ang = 2 * np.pi * (yy - ym[None, :]) / N
    U = np.ones((N, NC))
    for f in range(1, NF // 2 + 1):
        U += 2.0 * np.cos(f * ang)
    U /= NC                                                           # [1024,70]
    return atp, ctr, cti, U


# ---------------------------------------------------------------- bass program
def _build_program():
    import concourse.bass as bass
    import concourse.mybir as mybir
    import concourse.tile as tile
    from concourse import bacc

    f32 = mybir.dt.float32
    f16 = mybir.dt.float16
    bf16 = mybir.dt.bfloat16
    AF = mybir.ActivationFunctionType
    ALU = mybir.AluOpType

    nc = bacc.Bacc("TRN2", target_bir_lowering=False, debug=False)

    x_d = nc.dram_tensor("x", [N, N], bf16, kind="ExternalInput")
    atp_d = nc.dram_tensor("atp", [N, 2 * HK], bf16, kind="ExternalInput")
    # per-kernel transposed real/imag parts, indexed [v, (k,u)]
    ktr_d = nc.dram_tensor("ktr", [HK, K * HK], bf16, kind="ExternalInput")
    kti_d = nc.dram_tensor("kti", [HK, K * HK], bf16, kind="ExternalInput")
    # cc99: rows 0:35 = ctr, rows 64:99 = cti, rest zero  [99, 70]
    cc99_d = nc.dram_tensor("cc99", [99, NC], bf16, kind="ExternalInput")
    # ctl: cols 0:70 = [ctr;-cti], cols 70:140 = [cti;ctr]  [70, 140]
    ctl_d = nc.dram_tensor("ctl", [2 * HK, 2 * NC], bf16, kind="ExternalInput")
    # uc = [uht_half (512) | ut (1024)]  [70, 1536] fp16
    uc_d = nc.dram_tensor("uc", [NC, 1536], f16, kind="ExternalInput")

    aer_d = nc.dram_tensor("aer16", [512, N], f16, kind="ExternalOutput")
    res_d = nc.dram_tensor("res16", [512, N], f16, kind="ExternalOutput")
    prn_d = nc.dram_tensor("prn16", [512, N], f16, kind="ExternalOutput")

    with tile.TileContext(nc) as tc:
        with (
            tc.tile_pool(name="const", bufs=1) as cpool,
            tc.tile_pool(name="xin", bufs=4) as xpool,
            tc.tile_pool(name="work", bufs=1) as wpool,
            tc.tile_pool(name="sq", bufs=8) as sqpool,
            tc.tile_pool(name="outp", bufs=6) as opool,
        ):
            # ---- warmup tile (no DMA dep) to ramp the PE p-state ----
            warm_sb = cpool.tile([128, 128], bf16)
            nc.vector.memset(warm_sb[:], 0.0)

            # ---- const DMAs on gpsimd queue; x chunks on sync queue ----
            atp_sb = cpool.tile([128, 8, 2 * HK], bf16)
            nc.gpsimd.dma_start(
                atp_sb[:], atp_d.ap().rearrange("(c p) u -> p c u", p=128))

            x_sb = [xpool.tile([128, 2, N], bf16, tag="x", name=f"x{i}")
                    for i in range(4)]
            xr = x_d.ap().rearrange("(c p) w -> p c w", p=128)
            for i in range(4):
                nc.sync.dma_start(x_sb[i][:], xr[:, 2 * i:2 * i + 2, :])

            # kc1 stack [99+, 840]: ktR at partitions 0:35, ktI at 64:99
            kc1_sb = cpool.tile([128, K * HK], bf16)
            nc.gpsimd.dma_start(kc1_sb[0:HK, :], ktr_d[:, :])
            nc.gpsimd.dma_start(kc1_sb[64:64 + HK, :], kti_d[:, :])
            cc99_sb = cpool.tile([99, NC], bf16)
            nc.gpsimd.dma_start(cc99_sb[:], cc99_d[:, :])
            ctl_sb = cpool.tile([2 * HK, 2 * NC], bf16)
            nc.gpsimd.dma_start(ctl_sb[:], ctl_d[:, :])
            uc_sb = cpool.tile([NC, 1536], f16)
            nc.gpsimd.dma_start(uc_sb[:], uc_d[:, :])
            sig_bias = cpool.tile([128, 1], f32)
            nc.vector.memset(sig_bias[:], -RESIST_STEEPNESS * RESIST_THRESHOLD)

            # ---- PE warmup: ramp to full clock during the x DMA ----
            with tc.tile_pool(name="warm_ps", bufs=1,
                              space=bass.MemorySpace.PSUM) as wps:
                wp = wps.tile([128, 512], f32)
                for r in range(8):
                    nc.tensor.matmul(wp[:], warm_sb[:],
                                     warm_sb[:].unsqueeze(1)
                                     .broadcast_to([128, 4, 128]),
                                     start=True, stop=True)

            vcopy = lambda out, in_: nc.vector.tensor_scalar_mul(out, in_, 1.0)
            p1t_sb = wpool.tile([128, 8, 2 * HK], bf16)   # P1^T chunks

            # ---- stage 1: P1T[j,u] = sum_y x[y,j] * atp[y,u] ----
            with tc.tile_pool(name="p1t_ps", bufs=8,
                              space=bass.MemorySpace.PSUM) as p1ps:
                p1t_ps = [p1ps.tile([128, 2 * HK], f32, tag="p1t",
                                    name=f"p1t_ps{i}") for i in range(8)]
                for yc in range(8):
                    for jc in range(8):
                        nc.tensor.matmul(
                            p1t_ps[jc][:, :],
                            x_sb[yc // 2][:, yc % 2, jc * 128:(jc + 1) * 128],
                            atp_sb[:, yc, :],
                            start=(yc == 0), stop=(yc == 7),
                        )
                for jc in range(8):
                    if jc % 2 == 0:
                        nc.scalar.copy(p1t_sb[:, jc, :], p1t_ps[jc][:, :])
                    else:
                        vcopy(p1t_sb[:, jc, :], p1t_ps[jc][:, :])

            # ---- stage 1b: m4a = Ar@P1T, m4b = Ai@P1T  (contract over j) ----
            m4_sb = wpool.tile([HK, 2, 2 * HK], f32)
            with tc.tile_pool(name="m4_ps", bufs=2,
                              space=bass.MemorySpace.PSUM) as m4ps:
                m4a = m4ps.tile([HK, 2 * HK], f32)
                m4b = m4ps.tile([HK, 2 * HK], f32)
                for jc in range(8):
                    nc.tensor.matmul(m4a[:, :], atp_sb[:, jc, 0:HK],
                                     p1t_sb[:, jc, :],
                                     start=(jc == 0), stop=(jc == 7))
                    nc.tensor.matmul(m4b[:, :], atp_sb[:, jc, HK:2 * HK],
                                     p1t_sb[:, jc, :],
                                     start=(jc == 0), stop=(jc == 7))
                nc.scalar.copy(m4_sb[:, 0, :], m4a[:])
                nc.scalar.copy(m4_sb[:, 1, :], m4b[:])

            # 99-row stacks (imag half at partition 64 for 32-alignment):
            # mh2 = [MhT_r;;MhT_i], mh2s = [MhT_i;;MhT_r], mh2sn = [-MhT_i;;MhT_r]
            mh2 = wpool.tile([128, HK], bf16)
            mh2s = wpool.tile([128, HK], bf16)
            # MhT_r = m4a[:,0:35] - m4b[:,35:70]; MhT_i = m4a[:,35:70] + m4b[:,0:35]
            nc.vector.tensor_sub(mh2[0:HK, :], m4_sb[:, 0, 0:HK],
                                 m4_sb[:, 1, HK:2 * HK])
            nc.vector.tensor_add(mh2[64:64 + HK, :], m4_sb[:, 0, HK:2 * HK],
                                 m4_sb[:, 1, 0:HK])
            nc.scalar.copy(mh2s[0:HK, :], mh2[64:64 + HK, :])
            nc.scalar.copy(mh2s[64:64 + HK, :], mh2[0:HK, :])

            # ---- stage 2a: half-muls with same-base inputs ----
            # s1 = MhT_r*ktR, s2 = MhT_i*ktI, s3 = MhT_r*ktI, s4 = MhT_i*ktR
            kcv = lambda r0: kc1_sb[r0:r0 + HK, :].rearrange(
                "q (k u) -> q k u", k=K)
            mhb = lambda t, r0: t[r0:r0 + HK, :].unsqueeze(1).broadcast_to(
                [HK, K, HK])
            s1 = wpool.tile([HK, K * HK], bf16)
            s2 = wpool.tile([HK, K * HK], bf16)
            s3 = wpool.tile([HK, K * HK], bf16)
            s4 = wpool.tile([HK, K * HK], bf16)
            r3 = lambda t: t[:].rearrange("q (k u) -> q k u", k=K)
            nc.vector.tensor_mul(r3(s1), mhb(mh2, 0), kcv(0))
            nc.gpsimd.tensor_mul(r3(s2), mhb(mh2, 64), kcv(64))
            nc.vector.tensor_mul(r3(s3), mhb(mh2s, 64), kcv(64))
            nc.gpsimd.tensor_mul(r3(s4), mhb(mh2s, 0), kcv(0))

            # gt70b [99, K*70]: rows 0:35 = [GrT | GiT], rows 64:99 = [-GiT | GrT]
            gt = wpool.tile([128, K * 2 * HK], bf16)
            nc.vector.memset(gt[32:64, :], 0.0)
            gtv = lambda r0, c0: gt[r0:r0 + HK, :].rearrange(
                "q (k c) -> q k c", k=K)[:, :, c0:c0 + HK]
            # GrT = s1 - s2 -> (0:35, u); copied to (64:99, 35+u)
            nc.vector.tensor_sub(gtv(0, 0), r3(s1), r3(s2))
            nc.gpsimd.tensor_scalar_mul(gtv(64, HK), gtv(0, 0), 1.0)
            # GiT = s3 + s4 -> (0:35, 35+u); negated to (64:99, u)
            nc.vector.tensor_add(gtv(0, HK), r3(s3), r3(s4))
            nc.gpsimd.tensor_scalar_mul(gtv(64, 0), gtv(0, HK), -1.0)

            # ---- stage 2c: W-stacks, 6 kernels batched per PSUM group ----
            w_sb = wpool.tile([2 * HK, KG, KPG * NC], bf16)   # [70, 4, 420]
            # ---- stage 2d + squares + accumulation ----
            aer_c = wpool.tile([NC, NC], f32)
            aer16 = wpool.tile([NC, NC], f16)
            sgrp = []
            with (
                tc.tile_pool(name="w_ps", bufs=4, space=bass.MemorySpace.PSUM) as wps2,
                tc.tile_pool(name="f_ps", bufs=4, space=bass.MemorySpace.PSUM) as fps,
            ):
                for g in range(KG):
                    wgp = wps2.tile([2 * HK, KPG * NC], f32, tag="wg",
                                    name=f"wg{g}")
                    for j in range(KPG):
                        k = KPG * g + j
                        nc.tensor.matmul(wgp[:, j * NC:(j + 1) * NC],
                                         gt[0:99, k * 2 * HK:(k + 1) * 2 * HK],
                                         cc99_sb[:], start=True, stop=True)
                    nc.scalar.copy(w_sb[:, g, :], wgp[:])

                for g in range(KG):
                    fr = fps.tile([NC, KPG * NC], f32, tag="f", name=f"fr{g}")
                    fi = fps.tile([NC, KPG * NC], f32, tag="f", name=f"fi{g}")
                    nc.tensor.matmul(fr[:], ctl_sb[:, 0:NC], w_sb[:, g, :],
                                     start=True, stop=True)
                    nc.tensor.matmul(fi[:], ctl_sb[:, NC:2 * NC], w_sb[:, g, :],
                                     start=True, stop=True)
                    sq_r = sqpool.tile([NC, KPG * NC], bf16, tag="sq",
                                       name=f"sqr{g}")
                    sq_i = sqpool.tile([NC, KPG * NC], bf16, tag="sq",
                                       name=f"sqi{g}")
                    if g < 3:
                        nc.scalar.activation(sq_r[:], fr[:], AF.Square)
                        nc.scalar.activation(sq_i[:], fi[:], AF.Square)
                    else:
                        # DVE path: copy PSUM->SBUF bf16, then square-mul
                        cr = sqpool.tile([NC, KPG * NC], bf16, tag="sq",
                                         name="cr")
                        ci = sqpool.tile([NC, KPG * NC], bf16, tag="sq",
                                         name="ci")
                        vcopy(cr[:], fr[:])
                        vcopy(ci[:], fi[:])
                        nc.vector.tensor_mul(sq_r[:], cr[:], cr[:])
                        nc.vector.tensor_mul(sq_i[:], ci[:], ci[:])
                    s_g = sqpool.tile([NC, KPG * NC], bf16, tag="sacc",
                                      name=f"s{g}")
                    eng = nc.vector if g % 2 == 0 else nc.gpsimd
                    eng.tensor_add(s_g[:], sq_r[:], sq_i[:])
                    sgrp.append(s_g)

            s01 = sqpool.tile([NC, KPG * NC], bf16, tag="sacc2", name="s01")
            s23 = sqpool.tile([NC, KPG * NC], bf16, tag="sacc2", name="s23")
            stot = sqpool.tile([NC, KPG * NC], bf16, tag="sacc2", name="stot")
            nc.vector.tensor_add(s01[:], sgrp[0][:], sgrp[1][:])
            nc.gpsimd.tensor_add(s23[:], sgrp[2][:], sgrp[3][:])
            nc.vector.tensor_add(stot[:], s01[:], s23[:])
            # fold the KPG k-slices: [70, (k m)] -> reduce over k
            nc.vector.tensor_reduce(
                aer_c[:], stot[:].rearrange("p (k m) -> p m k", k=KPG),
                mybir.AxisListType.X, ALU.add)
            vcopy(aer16[:], aer_c[:])

            # ---- stage 5: aerial_half = U_h @ aer_c @ U^T  (fp16 matmuls) ----
            z_sb = wpool.tile([NC, 512], f16)
            with tc.tile_pool(name="z_ps", bufs=1,
                              space=bass.MemorySpace.PSUM) as zps:
                zp = zps.tile([NC, 512], f32)
                nc.tensor.matmul(zp[:], aer16[:], uc_sb[:, 0:512],
                                 start=True, stop=True)
                vcopy(z_sb[:], zp[:])

            with tc.tile_pool(name="a_ps", bufs=2,
                              space=bass.MemorySpace.PSUM) as aps:
                for t in range(4):
                    ap_t = aps.tile([128, N], f32)
                    for j in range(2):
                        nc.tensor.matmul(ap_t[:, j * 512:(j + 1) * 512],
                                         z_sb[:, t * 128:(t + 1) * 128],
                                         uc_sb[:, 512 + j * 512:512 + (j + 1) * 512],
                                         start=True, stop=True)
                    aer_sb = opool.tile([128, N], f16, tag="out", name="aer_sb")
                    res_sb = opool.tile([128, N], f16, tag="out", name="res_sb")
                    prn_sb = opool.tile([128, N], f16, tag="out", name="prn_sb")
                    nc.scalar.activation(res_sb[:], ap_t[:], AF.Sigmoid,
                                         bias=sig_bias[:],
                                         scale=RESIST_STEEPNESS)
                    vcopy(aer_sb[:], ap_t[:])
                    nc.gpsimd.tensor_scalar(prn_sb[:], res_sb[:], 0.5, None,
                                            op0=ALU.is_gt)
                    nc.sync.dma_start(aer_d[t * 128:(t + 1) * 128, :], aer_sb[:])
                    nc.scalar.dma_start(res_d[t * 128:(t + 1) * 128, :], res_sb[:])
                    nc.gpsimd.dma_start(prn_d[t * 128:(t + 1) * 128, :], prn_sb[:])

    nc.compile()
    return nc


_CACHE = {}


def _get_program():
    if "nc" not in _CACHE:
        _CACHE["nc"] = _build_program()
    return _CACHE["nc"]


def _prep_inputs(mask, kernels, scales):
    import ml_dtypes
    bf = ml_dtypes.bfloat16

    atp, ctr, cti, U = _host_matrices()

    kers = kernels.astype(np.complex128) * np.sqrt(
        scales.astype(np.float64))[:, None, None]
    # ktR/ktI indexed [v, (k,u)] = transposed per-kernel matrices
    ktR = kers.real.transpose(2, 0, 1).reshape(HK, K * HK).astype(bf)
    ktI = kers.imag.transpose(2, 0, 1).reshape(HK, K * HK).astype(bf)
    cc99 = np.zeros((99, NC), np.float32)
    cc99[0:HK] = ctr
    cc99[64:99] = cti
    cc99 = cc99.astype(bf)
    ctl = np.concatenate(
        [np.concatenate([ctr, -cti], axis=0),
         np.concatenate([cti, ctr], axis=0)], axis=1).astype(bf)  # [70, 140]
    ut = U.T                                                      # [70, 1024]
    uc = [np.concatenate([U[h * 512:(h + 1) * 512, :].T, ut],
                         axis=1).astype(np.float16) for h in range(2)]
    atp_bf = atp.astype(bf)
    mask_bf = np.asarray(mask, np.float32).astype(bf)
    return mask_bf, atp_bf, ktR, ktI, cc99, ctl, uc


# ---------------------------------------------------------------- entry point
def kernel(mask, kernels, kernels_ct, scales):
    """Full inputs in, full outputs out.  Shards over 8 NeuronCores internally."""
    from concourse.bass_utils import run_bass_kernel_spmd

    kernels = np.asarray(kernels, np.complex64)
    scales = np.asarray(scales, np.float32)
    mask_bf, atp_bf, ktR, ktI, cc99, ctl, uc = _prep_inputs(mask, kernels, scales)

    nc = _get_program()
    in_maps = []
    for c in range(8):
        b, h = c // 2, c % 2
        in_maps.append({
            "x": mask_bf[b],
            "atp": atp_bf,
            "ktr": ktR,
            "kti": ktI,
            "cc99": cc99,
            "ctl": ctl,
            "uc": uc[h],
        })

    trace = bool(int(os.environ.get("BASS_KERNEL_TRACE", "0")))
    res = run_bass_kernel_spmd(nc, in_maps, core_ids=list(range(8)), trace=trace)
    _CACHE["last_results"] = res

    aerial = np.empty((B, N, N), np.float32)
    resist = np.empty((B, N, N), np.float32)
    printed = np.empty((B, N, N), np.float32)
    for c in range(8):
        b, h = c // 2, c % 2
        rows = slice(h * 512, (h + 1) * 512)
        aerial[b, rows, :] = res.results[c]["aer16"].astype(np.float32)
        resist[b, rows, :] = res.results[c]["res16"].astype(np.float32)
        printed[b, rows, :] = res.results[c]["prn16"].astype(np.float32)
    return aerial, resist, printed


# revision 7
# speedup vs baseline: 2.0836x; 2.0836x over previous
"""Trainium2 Bass kernel for the SOCS lithography simulator.

Reference math (per batch b):
    aerial = sum_k s_k * | cIFFT2( cFFT2(mask_b) * pad_center(kernels[k]) ) |^2
    resist = sigmoid(50*(aerial - 0.225));  printed = (resist > 0.5)

The padded kernels live in the *frequency* domain with only a 35x35 window of
nonzero coefficients, so every field is band-limited to 35x35 frequencies and
aerial (a sum of |field|^2) is band-limited to 69x69.  Everything reduces to
small dense matmuls on a coarse grid of NC=70 >= 69 uniform samples per axis:

    Mhat  = A @ x @ A.T           A = rows 494:529 of the centered DFT matrix
    G_k   = Mhat * (sqrt(s_k) * kernels[k])                  [35,35] cplx
    W_k   = G_k @ C.T             C = inverse-DFT at y_m = 1024*m/70  [70,35]
    F_k   = C @ W_k               fields on the 70x70 coarse grid
    aer_c = sum_k |F_k|^2         exact coarse samples of aerial
    aerial = U @ aer_c @ U.T      U real [1024,70] Dirichlet interp (exact)

Complex products are folded into PE contractions via row-stacking:
  * stage 2c: lhsT = gt70b[99,70] per k (rows v / 64+v carry [GrT|GiT] and
    [-GiT|GrT]); rhs = cc99 [99,70] (rows ctr / cti) -> out [Wr;Wi] [70,70].
  * stage 2d: lhsT = [ctr;-cti] or [cti;ctr] [70,70]; rhs = W-stack with
    6 kernels batched in columns -> F_r/F_i [70,420] per group.

Precision: bf16 through stage 2, fp16 interpolation + fp16 outputs
(host upcasts to f32).  Simulated end-to-end rel l2 ~3.3e-3.

Sharding: 8 cores; core c handles batch c//2 and output row-half c%2.
No collectives.  Self-contained: shapes/constants hardcoded.
"""

import os

import numpy as np

N = 1024
B, K, HK = 4, 24, 35
PT = (N - HK) // 2          # 494
NC = 70                     # coarse grid (>= 2*HK-1 = 69)
NF = 2 * HK - 1             # 69 product frequencies
DOSE = 1.0
RESIST_THRESHOLD = 0.225
RESIST_STEEPNESS = 50.0
KG = 4                      # 2c/2d column-batched k-groups
KPG = K // KG               # kernels per group = 6


# ---------------------------------------------------------------- host matrices
def _host_matrices():
    """Input-independent constant matrices (f64 on host)."""
    u = np.arange(HK)[:, None]
    y = np.arange(N)[None, :]
    A = np.exp(-2j * np.pi * ((u + PT - N // 2) * (y - N // 2)) / N)  # [35,1024]
    atp = np.concatenate([A.real.T, A.imag.T], axis=1)                # [1024,70]

    m = np.arange(NC)
    ym = N * m / NC                                                   # fractional
    Ac = np.exp(-2j * np.pi * ((u + PT - N // 2) * (ym[None, :] - N // 2)) / N)
    C = np.conj(Ac).T / N                                             # [70,35]
    ctr = np.ascontiguousarray(C.real.T)                              # [35,70]
    cti = np.ascontiguousarray(C.imag.T)

    yy = np.arange(N)[:, None]
    ang = 2 * np.pi * (yy - ym[None, :]) / N
    U = np.ones((N, NC))
    for f in range(1, NF // 2 + 1):
        U += 2.0 * np.cos(f * ang)
    U /= NC                                                           # [1024,70]
    return atp, ctr, cti, U


# ---------------------------------------------------------------- bass program
def _build_program():
    import concourse.bass as bass
    import concourse.mybir as mybir
    import concourse.tile as tile
    from concourse import bacc

    f32 = mybir.dt.float32
    f16 = mybir.dt.float16
    bf16 = mybir.dt.bfloat16
    AF = mybir.ActivationFunctionType
    ALU = mybir.AluOpType

    nc = bacc.Bacc("TRN2", target_bir_lowering=False, debug=False)

    x_d = nc.dram_tensor("x", [N, N], bf16, kind="ExternalInput")
    atp_d = nc.dram_tensor("atp", [N, 2 * HK], bf16, kind="ExternalInput")
    # per-kernel transposed real/imag parts, indexed [v, (k,u)]
    ktr_d = nc.dram_tensor("ktr", [HK, K * HK], bf16, kind="ExternalInput")
    kti_d = nc.dram_tensor("kti", [HK, K * HK], bf16, kind="ExternalInput")
    # cc2 = [ctr | -cti]  [35, 140]
    cc2_d = nc.dram_tensor("cc2", [HK, 2 * NC], bf16, kind="ExternalInput")
    # ctl: cols 0:70 = [ctr;-cti], cols 70:140 = [cti;ctr]  [70, 140]
    ctl_d = nc.dram_tensor("ctl", [2 * HK, 2 * NC], bf16, kind="ExternalInput")
    # uc = [uht_half (512) | ut (1024)]  [70, 1536] fp16
    uc_d = nc.dram_tensor("uc", [NC, 1536], f16, kind="ExternalInput")

    aer_d = nc.dram_tensor("aer16", [512, N], f16, kind="ExternalOutput")
    res_d = nc.dram_tensor("res16", [512, N], f16, kind="ExternalOutput")
    prn_d = nc.dram_tensor("prn16", [512, N], f16, kind="ExternalOutput")

    with tile.TileContext(nc) as tc:
        with (
            tc.tile_pool(name="const", bufs=1) as cpool,
            tc.tile_pool(name="xin", bufs=4) as xpool,
            tc.tile_pool(name="work", bufs=1) as wpool,
            tc.tile_pool(name="sq", bufs=8) as sqpool,
            tc.tile_pool(name="outp", bufs=6) as opool,
        ):
            # ---- warmup tile (no DMA dep) to ramp the PE p-state ----
            warm_sb = cpool.tile([128, 128], bf16)
            nc.vector.memset(warm_sb[:], 0.0)

            # ---- const DMAs on gpsimd queue; x chunks on sync queue ----
            atp_sb = cpool.tile([128, 8, 2 * HK], bf16)
            nc.gpsimd.dma_start(
                atp_sb[:], atp_d.ap().rearrange("(c p) u -> p c u", p=128))

            x_sb = [xpool.tile([128, 2, N], bf16, tag="x", name=f"x{i}")
                    for i in range(4)]
            xr = x_d.ap().rearrange("(c p) w -> p c w", p=128)
            for i in range(4):
                nc.sync.dma_start(x_sb[i][:], xr[:, 2 * i:2 * i + 2, :])

            # kc1 stack [99+, 840]: ktR at partitions 0:35, ktI at 64:99
            kc1_sb = cpool.tile([128, K * HK], bf16)
            nc.gpsimd.dma_start(kc1_sb[0:HK, :], ktr_d[:, :])
            nc.gpsimd.dma_start(kc1_sb[64:64 + HK, :], kti_d[:, :])
            cc2_sb = cpool.tile([HK, 2 * NC], bf16)
            nc.gpsimd.dma_start(cc2_sb[:], cc2_d[:, :])
            ctl_sb = cpool.tile([2 * HK, 2 * NC], bf16)
            nc.gpsimd.dma_start(ctl_sb[:], ctl_d[:, :])
            uc_sb = cpool.tile([NC, 1536], f16)
            nc.gpsimd.dma_start(uc_sb[:], uc_d[:, :])
            sig_bias = cpool.tile([128, 1], f32)
            nc.vector.memset(sig_bias[:], -RESIST_STEEPNESS * RESIST_THRESHOLD)

            # ---- PE warmup: ramp to full clock during the x DMA ----
            with tc.tile_pool(name="warm_ps", bufs=1,
                              space=bass.MemorySpace.PSUM) as wps:
                wp = wps.tile([128, 512], f32)
                for r in range(8):
                    nc.tensor.matmul(wp[:], warm_sb[:],
                                     warm_sb[:].unsqueeze(1)
                                     .broadcast_to([128, 4, 128]),
                                     start=True, stop=True)

            vcopy = lambda out, in_: nc.vector.tensor_scalar_mul(out, in_, 1.0)
            p1t_sb = wpool.tile([128, 8, 2 * HK], bf16)   # P1^T chunks

            # ---- stage 1: P1T[j,u] = sum_y x[y,j] * atp[y,u] ----
            with tc.tile_pool(name="p1t_ps", bufs=8,
                              space=bass.MemorySpace.PSUM) as p1ps:
                p1t_ps = [p1ps.tile([128, 2 * HK], f32, tag="p1t",
                                    name=f"p1t_ps{i}") for i in range(8)]
                for yc in range(8):
                    for jc in range(8):
                        nc.tensor.matmul(
                            p1t_ps[jc][:, :],
                            x_sb[yc // 2][:, yc % 2, jc * 128:(jc + 1) * 128],
                            atp_sb[:, yc, :],
                            start=(yc == 0), stop=(yc == 7),
                        )
                for jc in range(8):
                    if jc % 2 == 0:
                        nc.scalar.copy(p1t_sb[:, jc, :], p1t_ps[jc][:, :])
                    else:
                        vcopy(p1t_sb[:, jc, :], p1t_ps[jc][:, :])

            # ---- stage 1b: m4a = Ar@P1T, m4b = Ai@P1T  (contract over j) ----
            m4_sb = wpool.tile([HK, 2, 2 * HK], f32)
            with tc.tile_pool(name="m4_ps", bufs=2,
                              space=bass.MemorySpace.PSUM) as m4ps:
                m4a = m4ps.tile([HK, 2 * HK], f32)
                m4b = m4ps.tile([HK, 2 * HK], f32)
                for jc in range(8):
                    nc.tensor.matmul(m4a[:, :], atp_sb[:, jc, 0:HK],
                                     p1t_sb[:, jc, :],
                                     start=(jc == 0), stop=(jc == 7))
                    nc.tensor.matmul(m4b[:, :], atp_sb[:, jc, HK:2 * HK],
                                     p1t_sb[:, jc, :],
                                     start=(jc == 0), stop=(jc == 7))
                nc.scalar.copy(m4_sb[:, 0, :], m4a[:])
                nc.scalar.copy(m4_sb[:, 1, :], m4b[:])

            # 99-row stacks (imag half at partition 64 for 32-alignment):
            # mh2 = [MhT_r;;MhT_i], mh2s = [MhT_i;;MhT_r], mh2sn = [-MhT_i;;MhT_r]
            mh2 = wpool.tile([128, HK], bf16)
            mh2s = wpool.tile([128, HK], bf16)
            # MhT_r = m4a[:,0:35] - m4b[:,35:70]; MhT_i = m4a[:,35:70] + m4b[:,0:35]
            nc.vector.tensor_sub(mh2[0:HK, :], m4_sb[:, 0, 0:HK],
                                 m4_sb[:, 1, HK:2 * HK])
            nc.vector.tensor_add(mh2[64:64 + HK, :], m4_sb[:, 0, HK:2 * HK],
                                 m4_sb[:, 1, 0:HK])
            nc.scalar.copy(mh2s[0:HK, :], mh2[64:64 + HK, :])
            nc.scalar.copy(mh2s[64:64 + HK, :], mh2[0:HK, :])

            # ---- stage 2a: half-muls (f32) + overlapped gt layout ----
            # s1 = MhT_r*ktR, s2 = MhT_i*ktI, s3 = MhT_r*ktI, s4 = MhT_i*ktR
            kcv = lambda r0: kc1_sb[r0:r0 + HK, :].rearrange(
                "q (k u) -> q k u", k=K)
            mhb = lambda t, r0: t[r0:r0 + HK, :].unsqueeze(1).broadcast_to(
                [HK, K, HK])
            s1 = wpool.tile([HK, K * HK], f32)
            s2 = wpool.tile([HK, K * HK], f32)
            s3 = wpool.tile([HK, K * HK], f32)
            s4 = wpool.tile([HK, K * HK], f32)
            r3 = lambda t: t[:].rearrange("q (k u) -> q k u", k=K)
            nc.vector.tensor_mul(r3(s1), mhb(mh2, 0), kcv(0))
            nc.gpsimd.tensor_mul(r3(s2), mhb(mh2, 64), kcv(64))
            nc.vector.tensor_mul(r3(s3), mhb(mh2s, 64), kcv(64))
            nc.gpsimd.tensor_mul(r3(s4), mhb(mh2s, 0), kcv(0))

            # gtAB [35, K*105]: per k [GrT | GiT | -GrT].  The two 2c lhsT
            # views overlap: gtA_k = cols 0:70 ([GrT|GiT]),
            # gtB_k = cols 35:105 ([GiT|-GrT]).
            gt = wpool.tile([HK, K * 3 * HK], bf16)
            gtv = lambda c0: gt[:, :].rearrange(
                "q (k c) -> q k c", k=K)[:, :, c0:c0 + HK]
            nc.vector.tensor_sub(gtv(0), r3(s1), r3(s2))       # GrT
            nc.vector.tensor_add(gtv(HK), r3(s3), r3(s4))      # GiT
            nc.gpsimd.tensor_sub(gtv(2 * HK), r3(s2), r3(s1))  # -GrT

            # ---- stage 2c: W-stacks, 6 kernels batched per PSUM group ----
            w_sb = wpool.tile([2 * HK, KG, KPG * NC], bf16)   # [70, 4, 420]
            # ---- stage 2d + squares + accumulation ----
            aer_c = wpool.tile([NC, NC], f32)
            aer16 = wpool.tile([NC, NC], f16)
            sgrp = []
            with (
                tc.tile_pool(name="w_ps", bufs=4, space=bass.MemorySpace.PSUM) as wps2,
                tc.tile_pool(name="f_ps", bufs=4, space=bass.MemorySpace.PSUM) as fps,
            ):
                for g in range(KG):
                    wgp = wps2.tile([2 * HK, KPG * NC], f32, tag="wg",
                                    name=f"wg{g}")
                    for j in range(KPG):
                        k = KPG * g + j
                        nc.tensor.matmul(wgp[:, j * NC:(j + 1) * NC],
                                         gt[:, k * 3 * HK:k * 3 * HK + 2 * HK],
                                         cc2_sb[:, 0:NC],
                                         start=True, stop=False)
                        nc.tensor.matmul(wgp[:, j * NC:(j + 1) * NC],
                                         gt[:, k * 3 * HK + HK:k * 3 * HK + 3 * HK],
                                         cc2_sb[:, NC:2 * NC],
                                         start=False, stop=True)
                    nc.scalar.copy(w_sb[:, g, :], wgp[:])

                for g in range(KG):
                    fr = fps.tile([NC, KPG * NC], f32, tag="f", name=f"fr{g}")
                    fi = fps.tile([NC, KPG * NC], f32, tag="f", name=f"fi{g}")
                    nc.tensor.matmul(fr[:], ctl_sb[:, 0:NC], w_sb[:, g, :],
                                     start=True, stop=True)
                    nc.tensor.matmul(fi[:], ctl_sb[:, NC:2 * NC], w_sb[:, g, :],
                                     start=True, stop=True)
                    sq_r = sqpool.tile([NC, KPG * NC], f32, tag="sq",
                                       name=f"sqr{g}")
                    sq_i = sqpool.tile([NC, KPG * NC], f32, tag="sq",
                                       name=f"sqi{g}")
                    if g < 3:
                        nc.scalar.activation(sq_r[:], fr[:], AF.Square)
                        nc.scalar.activation(sq_i[:], fi[:], AF.Square)
                    else:
                        # DVE path: copy PSUM->SBUF f32, then square-mul
                        cr = sqpool.tile([NC, KPG * NC], f32, tag="sq",
                                         name="cr")
                        ci = sqpool.tile([NC, KPG * NC], f32, tag="sq",
                                         name="ci")
                        vcopy(cr[:], fr[:])
                        vcopy(ci[:], fi[:])
                        nc.vector.tensor_mul(sq_r[:], cr[:], cr[:])
                        nc.vector.tensor_mul(sq_i[:], ci[:], ci[:])
                    s_g = sqpool.tile([NC, KPG * NC], f32, tag="sacc",
                                      name=f"s{g}")
                    eng = nc.vector if g % 2 == 0 else nc.gpsimd
                    eng.tensor_add(s_g[:], sq_r[:], sq_i[:])
                    sgrp.append(s_g)

            s01 = sqpool.tile([NC, KPG * NC], f32, tag="sacc2", name="s01")
            s23 = sqpool.tile([NC, KPG * NC], f32, tag="sacc2", name="s23")
            stot = sqpool.tile([NC, KPG * NC], f32, tag="sacc2", name="stot")
            nc.vector.tensor_add(s01[:], sgrp[0][:], sgrp[1][:])
            nc.gpsimd.tensor_add(s23[:], sgrp[2][:], sgrp[3][:])
            nc.vector.tensor_add(stot[:], s01[:], s23[:])
            # fold the KPG k-slices: [70, (k m)] -> reduce over k
            nc.vector.tensor_reduce(
                aer_c[:], stot[:].rearrange("p (k m) -> p m k", k=KPG),
                mybir.AxisListType.X, ALU.add)
            vcopy(aer16[:], aer_c[:])

            # ---- stage 5: aerial_half = U_h @ aer_c @ U^T  (fp16 matmuls) ----
            z_sb = wpool.tile([NC, 512], f16)
            with tc.tile_pool(name="z_ps", bufs=1,
                              space=bass.MemorySpace.PSUM) as zps:
                zp = zps.tile([NC, 512], f32)
                nc.tensor.matmul(zp[:], aer16[:], uc_sb[:, 0:512],
                                 start=True, stop=True)
                vcopy(z_sb[:], zp[:])

            with tc.tile_pool(name="a_ps", bufs=2,
                              space=bass.MemorySpace.PSUM) as aps:
                for t in range(4):
                    ap_t = aps.tile([128, N], f32)
                    for j in range(2):
                        nc.tensor.matmul(ap_t[:, j * 512:(j + 1) * 512],
                                         z_sb[:, t * 128:(t + 1) * 128],
                                         uc_sb[:, 512 + j * 512:512 + (j + 1) * 512],
                                         start=True, stop=True)
                    aer_sb = opool.tile([128, N], f16, tag="out", name="aer_sb")
                    res_sb = opool.tile([128, N], f16, tag="out", name="res_sb")
                    prn_sb = opool.tile([128, N], f16, tag="out", name="prn_sb")
                    nc.scalar.activation(res_sb[:], ap_t[:], AF.Sigmoid,
                                         bias=sig_bias[:],
                                         scale=RESIST_STEEPNESS)
                    if t % 2 == 0:
                        vcopy(aer_sb[:], ap_t[:])
                    else:
                        nc.scalar.copy(aer_sb[:], ap_t[:])
                    nc.vector.tensor_scalar(prn_sb[:], ap_t[:],
                                            RESIST_THRESHOLD, None,
                                            op0=ALU.is_gt)
                    nc.sync.dma_start(aer_d[t * 128:(t + 1) * 128, :], aer_sb[:])
                    nc.scalar.dma_start(res_d[t * 128:(t + 1) * 128, :], res_sb[:])
                    nc.gpsimd.dma_start(prn_d[t * 128:(t + 1) * 128, :], prn_sb[:])

    nc.compile()
    return nc


_CACHE = {}


def _get_program():
    if "nc" not in _CACHE:
        _CACHE["nc"] = _build_program()
    return _CACHE["nc"]


def _prep_inputs(mask, kernels, scales):
    import ml_dtypes
    bf = ml_dtypes.bfloat16

    atp, ctr, cti, U = _host_matrices()

    kers = kernels.astype(np.complex128) * np.sqrt(
        scales.astype(np.float64))[:, None, None]
    # ktR/ktI indexed [v, (k,u)] = transposed per-kernel matrices
    ktR = kers.real.transpose(2, 0, 1).reshape(HK, K * HK).astype(bf)
    ktI = kers.imag.transpose(2, 0, 1).reshape(HK, K * HK).astype(bf)
    cc2 = np.concatenate([ctr, -cti], axis=1).astype(bf)         # [35, 140]
    ctl = np.concatenate(
        [np.concatenate([ctr, -cti], axis=0),
         np.concatenate([cti, ctr], axis=0)], axis=1).astype(bf)  # [70, 140]
    ut = U.T                                                      # [70, 1024]
    uc = [np.concatenate([U[h * 512:(h + 1) * 512, :].T, ut],
                         axis=1).astype(np.float16) for h in range(2)]
    atp_bf = atp.astype(bf)
    mask_bf = np.asarray(mask, np.float32).astype(bf)
    return mask_bf, atp_bf, ktR, ktI, cc2, ctl, uc


# ---------------------------------------------------------------- entry point
def kernel(mask, kernels, kernels_ct, scales):
    """Full inputs in, full outputs out.  Shards over 8 NeuronCores internally."""
    from concourse.bass_utils import run_bass_kernel_spmd

    kernels = np.asarray(kernels, np.complex64)
    scales = np.asarray(scales, np.float32)
    mask_bf, atp_bf, ktR, ktI, cc2, ctl, uc = _prep_inputs(mask, kernels, scales)

    nc = _get_program()
    in_maps = []
    for c in range(8):
        b, h = c // 2, c % 2
        in_maps.append({
            "x": mask_bf[b],
            "atp": atp_bf,
            "ktr": ktR,
            "kti": ktI,
            "cc2": cc2,
            "ctl": ctl,
            "uc": uc[h],
        })

    trace = bool(int(os.environ.get("BASS_KERNEL_TRACE", "0")))
    res = run_bass_kernel_spmd(nc, in_maps, core_ids=list(range(8)), trace=trace)
    _CACHE["last_results"] = res

    aerial = np.empty((B, N, N), np.float32)
    resist = np.empty((B, N, N), np.float32)
    printed = np.empty((B, N, N), np.float32)
    for c in range(8):
        b, h = c // 2, c % 2
        rows = slice(h * 512, (h + 1) * 512)
        aerial[b, rows, :] = res.results[c]["aer16"].astype(np.float32)
        resist[b, rows, :] = res.results[c]["res16"].astype(np.float32)
        printed[b, rows, :] = res.results[c]["prn16"].astype(np.float32)
    return aerial, resist, printed


# revision 9
# speedup vs baseline: 2.1177x; 1.0164x over previous
"""Trainium2 Bass kernel for the SOCS lithography simulator.

Reference math (per batch b):
    aerial = sum_k s_k * | cIFFT2( cFFT2(mask_b) * pad_center(kernels[k]) ) |^2
    resist = sigmoid(50*(aerial - 0.225));  printed = (resist > 0.5)

The padded kernels live in the *frequency* domain with only a 35x35 window of
nonzero coefficients, so every field is band-limited to 35x35 frequencies and
aerial (a sum of |field|^2) is band-limited to 69x69.  Everything reduces to
small dense matmuls on a coarse grid of NC=70 >= 69 uniform samples per axis:

    Mhat  = A @ x @ A.T           A = rows 494:529 of the centered DFT matrix
    G_k   = Mhat * (sqrt(s_k) * kernels[k])                  [35,35] cplx
    W_k   = G_k @ C.T             C = inverse-DFT at y_m = 1024*m/70  [70,35]
    F_k   = C @ W_k               fields on the 70x70 coarse grid
    aer_c = sum_k |F_k|^2         exact coarse samples of aerial
    aerial = U @ aer_c @ U.T      U real [1024,70] Dirichlet interp (exact)

Complex products are folded into PE contractions via row-stacking:
  * stage 2c: lhsT = gt70b[99,70] per k (rows v / 64+v carry [GrT|GiT] and
    [-GiT|GrT]); rhs = cc99 [99,70] (rows ctr / cti) -> out [Wr;Wi] [70,70].
  * stage 2d: lhsT = [ctr;-cti] or [cti;ctr] [70,70]; rhs = W-stack with
    6 kernels batched in columns -> F_r/F_i [70,420] per group.

Precision: bf16 through stage 2, fp16 interpolation + fp16 outputs
(host upcasts to f32).  Simulated end-to-end rel l2 ~3.3e-3.

Sharding: 8 cores; core c handles batch c//2 and output row-half c%2.
No collectives.  Self-contained: shapes/constants hardcoded.
"""

import os

import numpy as np

N = 1024
B, K, HK = 4, 24, 35
PT = (N - HK) // 2          # 494
NC = 70                     # coarse grid (>= 2*HK-1 = 69)
NF = 2 * HK - 1             # 69 product frequencies
DOSE = 1.0
RESIST_THRESHOLD = 0.225
RESIST_STEEPNESS = 50.0
KG = 4                      # 2c/2d column-batched k-groups
KPG = K // KG               # kernels per group = 6


# ---------------------------------------------------------------- host matrices
def _host_matrices():
    """Input-independent constant matrices (f64 on host)."""
    u = np.arange(HK)[:, None]
    y = np.arange(N)[None, :]
    A = np.exp(-2j * np.pi * ((u + PT - N // 2) * (y - N // 2)) / N)  # [35,1024]
    atp = np.concatenate([A.real.T, A.imag.T], axis=1)                # [1024,70]

    m = np.arange(NC)
    ym = N * m / NC                                                   # fractional
    Ac = np.exp(-2j * np.pi * ((u + PT - N // 2) * (ym[None, :] - N // 2)) / N)
    C = np.conj(Ac).T / N                                             # [70,35]
    ctr = np.ascontiguousarray(C.real.T)                              # [35,70]
    cti = np.ascontiguousarray(C.imag.T)

    yy = np.arange(N)[:, None]
    ang = 2 * np.pi * (yy - ym[None, :]) / N
    U = np.ones((N, NC))
    for f in range(1, NF // 2 + 1):
        U += 2.0 * np.cos(f * ang)
    U /= NC                                                           # [1024,70]
    return atp, ctr, cti, U


# ---------------------------------------------------------------- bass program
def _build_program():
    import concourse.bass as bass
    import concourse.mybir as mybir
    import concourse.tile as tile
    from concourse import bacc

    f32 = mybir.dt.float32
    f16 = mybir.dt.float16
    bf16 = mybir.dt.bfloat16
    AF = mybir.ActivationFunctionType
    ALU = mybir.AluOpType

    nc = bacc.Bacc("TRN2", target_bir_lowering=False, debug=False)

    x_d = nc.dram_tensor("x", [N, N], bf16, kind="ExternalInput")
    atp_d = nc.dram_tensor("atp", [N, 2 * HK], bf16, kind="ExternalInput")
    # per-kernel transposed real/imag parts, indexed [v, (k,u)]
    ktr_d = nc.dram_tensor("ktr", [HK, K * HK], bf16, kind="ExternalInput")
    kti_d = nc.dram_tensor("kti", [HK, K * HK], bf16, kind="ExternalInput")
    # cc3 = [ctr | -cti | cti]  [35, 210]
    cc3_d = nc.dram_tensor("cc3", [HK, 3 * NC], bf16, kind="ExternalInput")
    # ctl99: cols 0:70 = [ctr;;-cti], cols 70:140 = [cti;;ctr]  [99, 140]
    ctl_d = nc.dram_tensor("ctl", [99, 2 * NC], bf16, kind="ExternalInput")
    # uc = [uht_half (512) | ut (1024)]  [70, 1536] fp16
    uc_d = nc.dram_tensor("uc", [NC, 1536], f16, kind="ExternalInput")

    aer_d = nc.dram_tensor("aer16", [512, N], f16, kind="ExternalOutput")
    res_d = nc.dram_tensor("res16", [512, N], f16, kind="ExternalOutput")
    prn_d = nc.dram_tensor("prn16", [512, N], f16, kind="ExternalOutput")

    with tile.TileContext(nc) as tc:
        with (
            tc.tile_pool(name="const", bufs=1) as cpool,
            tc.tile_pool(name="xin", bufs=4) as xpool,
            tc.tile_pool(name="work", bufs=1) as wpool,
            tc.tile_pool(name="sq", bufs=8) as sqpool,
            tc.tile_pool(name="outp", bufs=6) as opool,
        ):
            # ---- warmup tile (no DMA dep) to ramp the PE p-state ----
            warm_sb = cpool.tile([128, 128], bf16)
            nc.vector.memset(warm_sb[:], 0.0)

            # ---- const DMAs on gpsimd queue; x chunks on sync queue ----
            atp_sb = cpool.tile([128, 8, 2 * HK], bf16)
            nc.gpsimd.dma_start(
                atp_sb[:], atp_d.ap().rearrange("(c p) u -> p c u", p=128))

            x_sb = [xpool.tile([128, 2, N], bf16, tag="x", name=f"x{i}")
                    for i in range(4)]
            xr = x_d.ap().rearrange("(c p) w -> p c w", p=128)
            for i in range(4):
                eng = nc.sync if i % 2 == 0 else nc.scalar
                eng.dma_start(x_sb[i][:], xr[:, 2 * i:2 * i + 2, :])

            # kc1 stack [99+, 840]: ktR at partitions 0:35, ktI at 64:99
            kc1_sb = cpool.tile([128, K * HK], bf16)
            nc.gpsimd.dma_start(kc1_sb[0:HK, :], ktr_d[:, :])
            nc.gpsimd.dma_start(kc1_sb[64:64 + HK, :], kti_d[:, :])
            cc3_sb = cpool.tile([HK, 3 * NC], bf16)
            nc.gpsimd.dma_start(cc3_sb[:], cc3_d[:, :])
            ctl_sb = cpool.tile([99, 2 * NC], bf16)
            nc.gpsimd.dma_start(ctl_sb[:], ctl_d[:, :])
            uc_sb = cpool.tile([NC, 1536], f16)
            nc.gpsimd.dma_start(uc_sb[:], uc_d[:, :])
            sig_bias = cpool.tile([128, 1], f32)
            nc.vector.memset(sig_bias[:], -RESIST_STEEPNESS * RESIST_THRESHOLD)

            # ---- PE warmup: ramp to full clock during the x DMA ----
            with tc.tile_pool(name="warm_ps", bufs=1,
                              space=bass.MemorySpace.PSUM) as wps:
                wp = wps.tile([128, 128], f32)
                for r in range(8):
                    nc.tensor.matmul(wp[:], warm_sb[:], warm_sb[:],
                                     start=True, stop=True)

            vcopy = lambda out, in_: nc.vector.tensor_scalar_mul(out, in_, 1.0)
            p1t_sb = wpool.tile([128, 8, 2 * HK], bf16)   # P1^T chunks

            # ---- stage 1: P1T[j,u] = sum_y x[y,j] * atp[y,u] ----
            with tc.tile_pool(name="p1t_ps", bufs=8,
                              space=bass.MemorySpace.PSUM) as p1ps:
                p1t_ps = [p1ps.tile([128, 2 * HK], f32, tag="p1t",
                                    name=f"p1t_ps{i}") for i in range(8)]
                for yc in range(8):
                    for jc in range(8):
                        nc.tensor.matmul(
                            p1t_ps[jc][:, :],
                            x_sb[yc // 2][:, yc % 2, jc * 128:(jc + 1) * 128],
                            atp_sb[:, yc, :],
                            start=(yc == 0), stop=(yc == 7),
                        )
                for jc in range(8):
                    if jc % 2 == 0:
                        nc.scalar.copy(p1t_sb[:, jc, :], p1t_ps[jc][:, :])
                    else:
                        vcopy(p1t_sb[:, jc, :], p1t_ps[jc][:, :])

            # ---- stage 1b: m4a = Ar@P1T, m4b = Ai@P1T  (contract over j) ----
            m4_sb = wpool.tile([HK, 2, 2 * HK], f32)
            with tc.tile_pool(name="m4_ps", bufs=2,
                              space=bass.MemorySpace.PSUM) as m4ps:
                m4a = m4ps.tile([HK, 2 * HK], f32)
                m4b = m4ps.tile([HK, 2 * HK], f32)
                for jc in range(8):
                    nc.tensor.matmul(m4a[:, :], atp_sb[:, jc, 0:HK],
                                     p1t_sb[:, jc, :],
                                     start=(jc == 0), stop=(jc == 7))
                    nc.tensor.matmul(m4b[:, :], atp_sb[:, jc, HK:2 * HK],
                                     p1t_sb[:, jc, :],
                                     start=(jc == 0), stop=(jc == 7))
                nc.scalar.copy(m4_sb[:, 0, :], m4a[:])
                nc.scalar.copy(m4_sb[:, 1, :], m4b[:])

            # 99-row stacks (imag half at partition 64 for 32-alignment):
            # mh2 = [MhT_r;;MhT_i], mh2s = [MhT_i;;MhT_r], mh2sn = [-MhT_i;;MhT_r]
            mh2 = wpool.tile([128, HK], f32)
            mh2s = wpool.tile([128, HK], f32)
            # MhT_r = m4a[:,0:35] - m4b[:,35:70]; MhT_i = m4a[:,35:70] + m4b[:,0:35]
            nc.vector.tensor_sub(mh2[0:HK, :], m4_sb[:, 0, 0:HK],
                                 m4_sb[:, 1, HK:2 * HK])
            nc.vector.tensor_add(mh2[64:64 + HK, :], m4_sb[:, 0, HK:2 * HK],
                                 m4_sb[:, 1, 0:HK])
            nc.scalar.copy(mh2s[0:HK, :], mh2[64:64 + HK, :])
            nc.scalar.copy(mh2s[64:64 + HK, :], mh2[0:HK, :])

            # ---- stage 2a: half-muls (f32) + overlapped gt layout ----
            # s1 = MhT_r*ktR, s2 = MhT_i*ktI, s3 = MhT_r*ktI, s4 = MhT_i*ktR
            kcv = lambda r0: kc1_sb[r0:r0 + HK, :].rearrange(
                "q (k u) -> q k u", k=K)
            mhb = lambda t, r0: t[r0:r0 + HK, :].unsqueeze(1).broadcast_to(
                [HK, K, HK])
            s1 = wpool.tile([HK, K * HK], f32)
            s2 = wpool.tile([HK, K * HK], f32)
            s3 = wpool.tile([HK, K * HK], f32)
            s4 = wpool.tile([HK, K * HK], f32)
            r3 = lambda t: t[:].rearrange("q (k u) -> q k u", k=K)
            nc.vector.tensor_mul(r3(s1), mhb(mh2, 0), kcv(0))
            nc.gpsimd.tensor_mul(r3(s2), mhb(mh2, 64), kcv(64))
            nc.vector.tensor_mul(r3(s3), mhb(mh2s, 64), kcv(64))
            nc.gpsimd.tensor_mul(r3(s4), mhb(mh2s, 0), kcv(0))

            # gt role-major [35, 2*840]: [all GrT | all GiT],
            # contiguous combine writes; 2c lhsT = 2D per-(role,k) slices.
            KW = K * HK
            gt = wpool.tile([HK, 2 * KW], bf16)
            nc.vector.tensor_sub(gt[:, 0:KW], s1[:], s2[:])            # GrT
            nc.vector.tensor_add(gt[:, KW:2 * KW], s3[:], s4[:])       # GiT

            # ---- stage 2c: W-stacks (Wr rows 0:35, Wi rows 64:99) ----
            w_sb = wpool.tile([99, KG, KPG * NC], bf16)       # [99, 4, 420]
            # ---- stage 2d + squares + accumulation ----
            aer_c = wpool.tile([NC, NC], f32)
            aer16 = wpool.tile([NC, NC], f16)
            sgrp = []
            with (
                tc.tile_pool(name="w_ps", bufs=4, space=bass.MemorySpace.PSUM) as wps2,
                tc.tile_pool(name="f_ps", bufs=4, space=bass.MemorySpace.PSUM) as fps,
            ):
                for g in range(KG):
                    wgp = wps2.tile([128, KPG * NC], f32, tag="wg",
                                    name=f"wg{g}")
                    for j in range(KPG):
                        k = KPG * g + j
                        gr_k = gt[:, k * HK:(k + 1) * HK]
                        gi_k = gt[:, KW + k * HK:KW + (k + 1) * HK]
                        oc = slice(j * NC, (j + 1) * NC)
                        # Wr = GrT^T@ctr + GiT^T@(-cti)   -> rows 0:35
                        nc.tensor.matmul(wgp[0:HK, oc], gr_k,
                                         cc3_sb[:, 0:NC],
                                         start=True, stop=False)
                        nc.tensor.matmul(wgp[0:HK, oc], gi_k,
                                         cc3_sb[:, NC:2 * NC],
                                         start=False, stop=True)
                        # Wi = GiT^T@ctr + GrT^T@cti      -> rows 64:99
                        nc.tensor.matmul(wgp[64:64 + HK, oc], gi_k,
                                         cc3_sb[:, 0:NC],
                                         start=True, stop=False)
                        nc.tensor.matmul(wgp[64:64 + HK, oc], gr_k,
                                         cc3_sb[:, 2 * NC:3 * NC],
                                         start=False, stop=True)
                    nc.scalar.copy(w_sb[:, g, :], wgp[0:99, :])

                for g in range(KG):
                    fr = fps.tile([NC, KPG * NC], f32, tag="f", name=f"fr{g}")
                    fi = fps.tile([NC, KPG * NC], f32, tag="f", name=f"fi{g}")
                    nc.tensor.matmul(fr[:], ctl_sb[:, 0:NC], w_sb[:, g, :],
                                     start=True, stop=True)
                    nc.tensor.matmul(fi[:], ctl_sb[:, NC:2 * NC], w_sb[:, g, :],
                                     start=True, stop=True)
                    sq_r = sqpool.tile([NC, KPG * NC], f32, tag="sq",
                                       name=f"sqr{g}")
                    sq_i = sqpool.tile([NC, KPG * NC], f32, tag="sq",
                                       name=f"sqi{g}")
                    if g < 3:
                        nc.scalar.activation(sq_r[:], fr[:], AF.Square)
                        nc.scalar.activation(sq_i[:], fi[:], AF.Square)
                    else:
                        # DVE path: copy PSUM->SBUF f32, then square-mul
                        cr = sqpool.tile([NC, KPG * NC], f32, tag="sq",
                                         name="cr")
                        ci = sqpool.tile([NC, KPG * NC], f32, tag="sq",
                                         name="ci")
                        vcopy(cr[:], fr[:])
                        vcopy(ci[:], fi[:])
                        nc.vector.tensor_mul(sq_r[:], cr[:], cr[:])
                        nc.vector.tensor_mul(sq_i[:], ci[:], ci[:])
                    s_g = sqpool.tile([NC, KPG * NC], f32, tag="sacc",
                                      name=f"s{g}")
                    eng = nc.vector if g % 2 == 0 else nc.gpsimd
                    eng.tensor_add(s_g[:], sq_r[:], sq_i[:])
                    sgrp.append(s_g)

            s01 = sqpool.tile([NC, KPG * NC], f32, tag="sacc2", name="s01")
            s23 = sqpool.tile([NC, KPG * NC], f32, tag="sacc2", name="s23")
            stot = sqpool.tile([NC, KPG * NC], f32, tag="sacc2", name="stot")
            nc.vector.tensor_add(s01[:], sgrp[0][:], sgrp[1][:])
            nc.gpsimd.tensor_add(s23[:], sgrp[2][:], sgrp[3][:])
            nc.vector.tensor_add(stot[:], s01[:], s23[:])
            # fold the KPG k-slices: [70, (k m)] -> reduce over k
            nc.vector.tensor_reduce(
                aer_c[:], stot[:].rearrange("p (k m) -> p m k", k=KPG),
                mybir.AxisListType.X, ALU.add)
            vcopy(aer16[:], aer_c[:])

            # ---- stage 5: aerial_half = U_h @ aer_c @ U^T  (fp16 matmuls) ----
            z_sb = wpool.tile([NC, 512], f16)
            with tc.tile_pool(name="z_ps", bufs=1,
                              space=bass.MemorySpace.PSUM) as zps:
                zp = zps.tile([NC, 512], f32)
                nc.tensor.matmul(zp[:], aer16[:], uc_sb[:, 0:512],
                                 start=True, stop=True)
                vcopy(z_sb[:], zp[:])

            with tc.tile_pool(name="a_ps", bufs=2,
                              space=bass.MemorySpace.PSUM) as aps:
                for t in range(4):
                    ap_t = aps.tile([128, N], f32)
                    for j in range(2):
                        nc.tensor.matmul(ap_t[:, j * 512:(j + 1) * 512],
                                         z_sb[:, t * 128:(t + 1) * 128],
                                         uc_sb[:, 512 + j * 512:512 + (j + 1) * 512],
                                         start=True, stop=True)
                    aer_sb = opool.tile([128, N], f16, tag="out", name="aer_sb")
                    res_sb = opool.tile([128, N], f16, tag="out", name="res_sb")
                    prn_sb = opool.tile([128, N], f16, tag="out", name="prn_sb")
                    nc.scalar.activation(res_sb[:], ap_t[:], AF.Sigmoid,
                                         bias=sig_bias[:],
                                         scale=RESIST_STEEPNESS)
                    if t % 2 == 0:
                        vcopy(aer_sb[:], ap_t[:])
                    else:
                        nc.scalar.copy(aer_sb[:], ap_t[:])
                    nc.vector.tensor_scalar(prn_sb[:], ap_t[:],
                                            RESIST_THRESHOLD, None,
                                            op0=ALU.is_gt)
                    nc.sync.dma_start(aer_d[t * 128:(t + 1) * 128, :], aer_sb[:])
                    nc.scalar.dma_start(res_d[t * 128:(t + 1) * 128, :], res_sb[:])
                    nc.gpsimd.dma_start(prn_d[t * 128:(t + 1) * 128, :], prn_sb[:])

    nc.compile()
    return nc


_CACHE = {}


def _get_program():
    if "nc" not in _CACHE:
        _CACHE["nc"] = _build_program()
    return _CACHE["nc"]


def _prep_inputs(mask, kernels, scales):
    import ml_dtypes
    bf = ml_dtypes.bfloat16

    atp, ctr, cti, U = _host_matrices()

    kers = kernels.astype(np.complex128) * np.sqrt(
        scales.astype(np.float64))[:, None, None]
    # ktR/ktI indexed [v, (k,u)] = transposed per-kernel matrices
    ktR = kers.real.transpose(2, 0, 1).reshape(HK, K * HK).astype(bf)
    ktI = kers.imag.transpose(2, 0, 1).reshape(HK, K * HK).astype(bf)
    cc3 = np.concatenate([ctr, -cti, cti], axis=1).astype(bf)    # [35, 210]
    ctl = np.zeros((99, 2 * NC), np.float32)
    ctl[0:HK, 0:NC] = ctr
    ctl[64:99, 0:NC] = -cti
    ctl[0:HK, NC:2 * NC] = cti
    ctl[64:99, NC:2 * NC] = ctr
    ctl = ctl.astype(bf)                                          # [99, 140]
    ut = U.T                                                      # [70, 1024]
    uc = [np.concatenate([U[h * 512:(h + 1) * 512, :].T, ut],
                         axis=1).astype(np.float16) for h in range(2)]
    atp_bf = atp.astype(bf)
    mask_bf = np.asarray(mask, np.float32).astype(bf)
    return mask_bf, atp_bf, ktR, ktI, cc3, ctl, uc


# ---------------------------------------------------------------- entry point
def kernel(mask, kernels, kernels_ct, scales):
    """Full inputs in, full outputs out.  Shards over 8 NeuronCores internally."""
    from concourse.bass_utils import run_bass_kernel_spmd

    kernels = np.asarray(kernels, np.complex64)
    scales = np.asarray(scales, np.float32)
    mask_bf, atp_bf, ktR, ktI, cc3, ctl, uc = _prep_inputs(mask, kernels, scales)

    nc = _get_program()
    in_maps = []
    for c in range(8):
        b, h = c // 2, c % 2
        in_maps.append({
            "x": mask_bf[b],
            "atp": atp_bf,
            "ktr": ktR,
            "kti": ktI,
            "cc3": cc3,
            "ctl": ctl,
            "uc": uc[h],
        })

    trace = bool(int(os.environ.get("BASS_KERNEL_TRACE", "0")))
    res = run_bass_kernel_spmd(nc, in_maps, core_ids=list(range(8)), trace=trace)
    _CACHE["last_results"] = res

    aerial = np.empty((B, N, N), np.float32)
    resist = np.empty((B, N, N), np.float32)
    printed = np.empty((B, N, N), np.float32)
    for c in range(8):
        b, h = c // 2, c % 2
        rows = slice(h * 512, (h + 1) * 512)
        aerial[b, rows, :] = res.results[c]["aer16"].astype(np.float32)
        resist[b, rows, :] = res.results[c]["res16"].astype(np.float32)
        printed[b, rows, :] = res.results[c]["prn16"].astype(np.float32)
    return aerial, resist, printed


# revision 11
# speedup vs baseline: 2.5348x; 1.1970x over previous
"""Trainium2 Bass kernel for the SOCS lithography simulator.

Reference math (per batch b):
    aerial = sum_k s_k * | cIFFT2( cFFT2(mask_b) * pad_center(kernels[k]) ) |^2
    resist = sigmoid(50*(aerial - 0.225));  printed = (resist > 0.5)

The padded kernels live in the *frequency* domain with only a 35x35 window of
nonzero coefficients, so every field is band-limited to 35x35 frequencies and
aerial (a sum of |field|^2) is band-limited to 69x69.  Everything reduces to
small dense matmuls on a coarse grid of NC=70 >= 69 uniform samples per axis:

    Mhat  = A @ x @ A.T           A = rows 494:529 of the centered DFT matrix
    G_k   = Mhat * (sqrt(s_k) * kernels[k])                  [35,35] cplx
    W_k   = G_k @ C.T             C = inverse-DFT at y_m = 1024*m/70  [70,35]
    F_k   = C @ W_k               fields on the 70x70 coarse grid
    aer_c = sum_k |F_k|^2         exact coarse samples of aerial
    aerial = U @ aer_c @ U.T      U real [1024,70] Dirichlet interp (exact)

Stage-2 complex products run as 4 small PE matmuls per kernel
(lhsT = GrT_k / GiT_k role slices, rhs = [ctr | -cti | cti] columns),
accumulating Wr into PSUM rows 0:35 and Wi into rows 64:99; stage 2d then
contracts all 99 rows against a zero-padded [99,140] C-matrix stack.

Precision: fp8(e4m3) mask + stage-1 DFT operands, bf16 stage 2, fp16
interpolation + fp16/fp8 outputs (host upcasts).  Measured rel l2 ~2e-3.

Sharding: 8 cores; core c handles batch c//2 and output row-half c%2.
No collectives.  Self-contained: shapes/constants hardcoded.
"""

import os

import numpy as np

N = 1024
B, K, HK = 4, 24, 35
PT = (N - HK) // 2          # 494
NC = 70                     # coarse grid (>= 2*HK-1 = 69)
NF = 2 * HK - 1             # 69 product frequencies
DOSE = 1.0
RESIST_THRESHOLD = 0.225
RESIST_STEEPNESS = 50.0
KG = 4                      # 2c/2d column-batched k-groups
KPG = K // KG               # kernels per group = 6
KW = K * HK                 # 840
KH = K // 2                 # k-half split for 2a->2c pipelining


# ---------------------------------------------------------------- host matrices
def _host_matrices():
    """Input-independent constant matrices (f64 on host)."""
    u = np.arange(HK)[:, None]
    y = np.arange(N)[None, :]
    A = np.exp(-2j * np.pi * ((u + PT - N // 2) * (y - N // 2)) / N)  # [35,1024]
    atp = np.concatenate([A.real.T, A.imag.T], axis=1)                # [1024,70]

    m = np.arange(NC)
    ym = N * m / NC                                                   # fractional
    Ac = np.exp(-2j * np.pi * ((u + PT - N // 2) * (ym[None, :] - N // 2)) / N)
    C = np.conj(Ac).T / N                                             # [70,35]
    ctr = np.ascontiguousarray(C.real.T)                              # [35,70]
    cti = np.ascontiguousarray(C.imag.T)

    yy = np.arange(N)[:, None]
    ang = 2 * np.pi * (yy - ym[None, :]) / N
    U = np.ones((N, NC))
    for f in range(1, NF // 2 + 1):
        U += 2.0 * np.cos(f * ang)
    U /= NC                                                           # [1024,70]
    return atp, ctr, cti, U


# ---------------------------------------------------------------- bass program
def _build_program():
    import concourse.bass as bass
    import concourse.mybir as mybir
    import concourse.tile as tile
    from concourse import bacc

    f32 = mybir.dt.float32
    f16 = mybir.dt.float16
    bf16 = mybir.dt.bfloat16
    f8 = mybir.dt.float8e4
    AF = mybir.ActivationFunctionType
    ALU = mybir.AluOpType

    nc = bacc.Bacc("TRN2", target_bir_lowering=False, debug=False)

    x_d = nc.dram_tensor("x", [N, N], f8, kind="ExternalInput")
    atp8_d = nc.dram_tensor("atp8", [N, 2 * HK], f8, kind="ExternalInput")
    atp_d = nc.dram_tensor("atp", [N, 2 * HK], bf16, kind="ExternalInput")
    # per-kernel transposed real/imag parts, indexed [v, (k,u)]
    ktr_d = nc.dram_tensor("ktr", [HK, K * HK], bf16, kind="ExternalInput")
    kti_d = nc.dram_tensor("kti", [HK, K * HK], bf16, kind="ExternalInput")
    # cc3 = [ctr | -cti | cti]  [35, 210]
    cc3_d = nc.dram_tensor("cc3", [HK, 3 * NC], bf16, kind="ExternalInput")
    # ctl99: cols 0:70 = [ctr;;-cti], cols 70:140 = [cti;;ctr]  [99, 140]
    ctl_d = nc.dram_tensor("ctl", [99, 2 * NC], bf16, kind="ExternalInput")
    # uc = [uht_half (512) | ut (1024)]  [70, 1536] fp16
    uc_d = nc.dram_tensor("uc", [NC, 1536], f16, kind="ExternalInput")

    aer_d = nc.dram_tensor("aer16", [512, N], f16, kind="ExternalOutput")
    res_d = nc.dram_tensor("res16", [512, N], f16, kind="ExternalOutput")
    prn_d = nc.dram_tensor("prn8", [512, N], f8, kind="ExternalOutput")

    with tile.TileContext(nc) as tc:
        with (
            tc.tile_pool(name="const", bufs=1) as cpool,
            tc.tile_pool(name="xin", bufs=4) as xpool,
            tc.tile_pool(name="work", bufs=1) as wpool,
            tc.tile_pool(name="sq", bufs=8) as sqpool,
            tc.tile_pool(name="outp", bufs=6) as opool,
        ):
            # ---- warmup tile (no DMA dep) to ramp the PE p-state ----
            warm_sb = cpool.tile([128, 128], bf16)
            nc.vector.memset(warm_sb[:], 0.0)
            sig_bias = cpool.tile([128, 1], f32)
            nc.vector.memset(sig_bias[:], -RESIST_STEEPNESS * RESIST_THRESHOLD)
            # preload the sigmoid act table off the critical path
            dummy = cpool.tile([128, 1], f32)
            nc.scalar.activation(dummy[:], sig_bias[:], AF.Sigmoid,
                                 bias=sig_bias[:], scale=RESIST_STEEPNESS)

            # ---- const DMAs on gpsimd queue; x chunks on sync+scalar ----
            atp8_sb = cpool.tile([128, 8, 2 * HK], f8)
            nc.gpsimd.dma_start(
                atp8_sb[:], atp8_d.ap().rearrange("(c p) u -> p c u", p=128))

            x_sb = [xpool.tile([128, 2, N], f8, tag="x", name=f"x{i}")
                    for i in range(4)]
            xr = x_d.ap().rearrange("(c p) w -> p c w", p=128)
            for i in range(4):
                eng = nc.sync if i % 2 == 0 else nc.scalar
                eng.dma_start(x_sb[i][:], xr[:, 2 * i:2 * i + 2, :])

            atp_sb = cpool.tile([128, 8, 2 * HK], bf16)
            nc.gpsimd.dma_start(
                atp_sb[:], atp_d.ap().rearrange("(c p) u -> p c u", p=128))
            ktr_sb = cpool.tile([HK, K * HK], bf16)
            nc.gpsimd.dma_start(ktr_sb[:], ktr_d[:, :])
            kti_sb = cpool.tile([HK, K * HK], bf16)
            nc.gpsimd.dma_start(kti_sb[:], kti_d[:, :])
            cc3_sb = cpool.tile([HK, 3 * NC], bf16)
            nc.gpsimd.dma_start(cc3_sb[:], cc3_d[:, :])
            ctl_sb = cpool.tile([99, 2 * NC], bf16)
            nc.gpsimd.dma_start(ctl_sb[:], ctl_d[:, :])
            uc_sb = cpool.tile([NC, 1536], f16)
            nc.gpsimd.dma_start(uc_sb[:], uc_d[:, :])

            # ---- PE warmup: ramp to full clock during the x DMA ----
            with tc.tile_pool(name="warm_ps", bufs=1,
                              space=bass.MemorySpace.PSUM) as wps:
                wp = wps.tile([128, 128], f32)
                for r in range(8):
                    nc.tensor.matmul(wp[:], warm_sb[:], warm_sb[:],
                                     start=True, stop=True)

            vcopy = lambda out, in_: nc.vector.tensor_scalar_mul(out, in_, 1.0)
            p1t_sb = wpool.tile([128, 8, 2 * HK], bf16)   # P1^T chunks

            # ---- stage 1: P1T[j,u] = sum_y x[y,j] * atp8[y,u]  (fp8) ----
            with tc.tile_pool(name="p1t_ps", bufs=8,
                              space=bass.MemorySpace.PSUM) as p1ps:
                p1t_ps = [p1ps.tile([128, 2 * HK], f32, tag="p1t",
                                    name=f"p1t_ps{i}") for i in range(8)]
                for yc in range(8):
                    for jc in range(8):
                        nc.tensor.matmul(
                            p1t_ps[jc][:, :],
                            x_sb[yc // 2][:, yc % 2, jc * 128:(jc + 1) * 128],
                            atp8_sb[:, yc, :],
                            start=(yc == 0), stop=(yc == 7),
                        )
                for jc in range(8):
                    if jc % 2 == 0:
                        nc.scalar.copy(p1t_sb[:, jc, :], p1t_ps[jc][:, :])
                    else:
                        vcopy(p1t_sb[:, jc, :], p1t_ps[jc][:, :])

            # ---- stage 1b: m4a = Ar@P1T, m4b = Ai@P1T  (contract over j) ----
            m4_sb = wpool.tile([HK, 2, 2 * HK], f32)
            with tc.tile_pool(name="m4_ps", bufs=2,
                              space=bass.MemorySpace.PSUM) as m4ps:
                m4a = m4ps.tile([HK, 2 * HK], f32)
                m4b = m4ps.tile([HK, 2 * HK], f32)
                for jc in range(8):
                    nc.tensor.matmul(m4a[:, :], atp_sb[:, jc, 0:HK],
                                     p1t_sb[:, jc, :],
                                     start=(jc == 0), stop=(jc == 7))
                    nc.tensor.matmul(m4b[:, :], atp_sb[:, jc, HK:2 * HK],
                                     p1t_sb[:, jc, :],
                                     start=(jc == 0), stop=(jc == 7))
                nc.scalar.copy(m4_sb[:, 0, :], m4a[:])
                nc.scalar.copy(m4_sb[:, 1, :], m4b[:])

            # MhT_r = m4a[:,0:35] - m4b[:,35:70]; MhT_i = m4a[:,35:70] + m4b[:,0:35]
            mhr = wpool.tile([HK, HK], f32)
            mhi = wpool.tile([HK, HK], f32)
            nc.vector.tensor_sub(mhr[:], m4_sb[:, 0, 0:HK],
                                 m4_sb[:, 1, HK:2 * HK])
            nc.vector.tensor_add(mhi[:], m4_sb[:, 0, HK:2 * HK],
                                 m4_sb[:, 1, 0:HK])

            # ---- stage 2a: products + role combines, split in k-halves ----
            # s1 = MhT_r*ktR, s2 = MhT_i*ktI, s3 = MhT_r*ktI, s4 = MhT_i*ktR
            # gt_h = [GrT-half | GiT-half]  ([35, 840] per k-half)
            HW_ = KH * HK                                    # 420 cols per half
            mhrb = mhr[:].unsqueeze(1).broadcast_to([HK, KH, HK])
            mhib = mhi[:].unsqueeze(1).broadcast_to([HK, KH, HK])
            kv = lambda t, h: t[:, h * HW_:(h + 1) * HW_].rearrange(
                "q (k u) -> q k u", k=KH)
            s_t = {}
            for nm in ("s1", "s2", "s3", "s4"):
                s_t[nm] = wpool.tile([HK, KW], f32, name=nm)
            sv = lambda nm, h: s_t[nm][:, h * HW_:(h + 1) * HW_].rearrange(
                "q (k u) -> q k u", k=KH)
            sc = lambda nm, h: s_t[nm][:, h * HW_:(h + 1) * HW_]
            gt_h = [wpool.tile([HK, 2 * HW_], bf16, tag="gt", name=f"gt{h}")
                    for h in range(2)]
            for h in range(2):
                nc.gpsimd.tensor_mul(sv("s2", h), mhib, kv(kti_sb, h))
                nc.vector.tensor_mul(sv("s1", h), mhrb, kv(ktr_sb, h))
                nc.vector.tensor_mul(sv("s3", h), mhrb, kv(kti_sb, h))
                nc.vector.tensor_mul(sv("s4", h), mhib, kv(ktr_sb, h))
                nc.vector.tensor_sub(gt_h[h][:, 0:HW_], sc("s1", h), sc("s2", h))
                nc.vector.tensor_add(gt_h[h][:, HW_:2 * HW_], sc("s3", h),
                                     sc("s4", h))

            # ---- stage 2c: W-stacks (Wr rows 0:35, Wi rows 64:99) ----
            w_sb = wpool.tile([99, KG, KPG * NC], bf16)       # [99, 4, 420]
            # ---- stage 2d + squares + accumulation ----
            aer_c = wpool.tile([NC, NC], f32)
            aer16 = wpool.tile([NC, NC], f16)
            sgrp = []
            with (
                tc.tile_pool(name="w_ps", bufs=4, space=bass.MemorySpace.PSUM) as wps2,
                tc.tile_pool(name="f_ps", bufs=4, space=bass.MemorySpace.PSUM) as fps,
            ):
                for g in range(KG):
                    wgp = wps2.tile([128, KPG * NC], f32, tag="wg",
                                    name=f"wg{g}")
                    for j in range(KPG):
                        k = KPG * g + j
                        h, kk = divmod(k, KH)
                        gr_k = gt_h[h][:, kk * HK:(kk + 1) * HK]
                        gi_k = gt_h[h][:, HW_ + kk * HK:HW_ + (kk + 1) * HK]
                        oc = slice(j * NC, (j + 1) * NC)
                        # Wr = GrT^T@ctr + GiT^T@(-cti)   -> rows 0:35
                        nc.tensor.matmul(wgp[0:HK, oc], gr_k,
                                         cc3_sb[:, 0:NC],
                                         start=True, stop=False)
                        nc.tensor.matmul(wgp[0:HK, oc], gi_k,
                                         cc3_sb[:, NC:2 * NC],
                                         start=False, stop=True)
                        # Wi = GiT^T@ctr + GrT^T@cti      -> rows 64:99
                        nc.tensor.matmul(wgp[64:64 + HK, oc], gi_k,
                                         cc3_sb[:, 0:NC],
                                         start=True, stop=False)
                        nc.tensor.matmul(wgp[64:64 + HK, oc], gr_k,
                                         cc3_sb[:, 2 * NC:3 * NC],
                                         start=False, stop=True)
                    nc.scalar.copy(w_sb[:, g, :], wgp[0:99, :])

                for g in range(KG):
                    fr = fps.tile([NC, KPG * NC], f32, tag="f", name=f"fr{g}")
                    fi = fps.tile([NC, KPG * NC], f32, tag="f", name=f"fi{g}")
                    nc.tensor.matmul(fr[:], ctl_sb[:, 0:NC], w_sb[:, g, :],
                                     start=True, stop=True)
                    nc.tensor.matmul(fi[:], ctl_sb[:, NC:2 * NC], w_sb[:, g, :],
                                     start=True, stop=True)
                    sq_r = sqpool.tile([NC, KPG * NC], f32, tag="sq",
                                       name=f"sqr{g}")
                    sq_i = sqpool.tile([NC, KPG * NC], f32, tag="sq",
                                       name=f"sqi{g}")
                    if g < 3:
                        nc.scalar.activation(sq_r[:], fr[:], AF.Square)
                        nc.scalar.activation(sq_i[:], fi[:], AF.Square)
                    else:
                        # DVE path: copy PSUM->SBUF f32, then square-mul
                        cr = sqpool.tile([NC, KPG * NC], f32, tag="sq",
                                         name="cr")
                        ci = sqpool.tile([NC, KPG * NC], f32, tag="sq",
                                         name="ci")
                        vcopy(cr[:], fr[:])
                        vcopy(ci[:], fi[:])
                        nc.vector.tensor_mul(sq_r[:], cr[:], cr[:])
                        nc.vector.tensor_mul(sq_i[:], ci[:], ci[:])
                    s_g = sqpool.tile([NC, KPG * NC], f32, tag="sacc",
                                      name=f"s{g}")
                    nc.vector.tensor_add(s_g[:], sq_r[:], sq_i[:])
                    sgrp.append(s_g)

            s01 = sqpool.tile([NC, KPG * NC], f32, tag="sacc2", name="s01")
            s23 = sqpool.tile([NC, KPG * NC], f32, tag="sacc2", name="s23")
            stot = sqpool.tile([NC, KPG * NC], f32, tag="sacc2", name="stot")
            nc.vector.tensor_add(s01[:], sgrp[0][:], sgrp[1][:])
            nc.vector.tensor_add(s23[:], sgrp[2][:], sgrp[3][:])
            nc.vector.tensor_add(stot[:], s01[:], s23[:])
            # fold the KPG k-slices: [70, (k m)] -> reduce over k
            nc.vector.tensor_reduce(
                aer_c[:], stot[:].rearrange("p (k m) -> p m k", k=KPG),
                mybir.AxisListType.X, ALU.add)
            vcopy(aer16[:], aer_c[:])

            # ---- stage 5: aerial_half = U_h @ aer_c @ U^T  (fp16 matmuls) ----
            z_sb = wpool.tile([NC, 512], f16)
            with tc.tile_pool(name="z_ps", bufs=1,
                              space=bass.MemorySpace.PSUM) as zps:
                zp = zps.tile([NC, 512], f32)
                nc.tensor.matmul(zp[:], aer16[:], uc_sb[:, 0:512],
                                 start=True, stop=True)
                vcopy(z_sb[:], zp[:])

            with tc.tile_pool(name="a_ps", bufs=2,
                              space=bass.MemorySpace.PSUM) as aps:
                for t in range(4):
                    ap_t = aps.tile([128, N], f32)
                    for j in range(2):
                        nc.tensor.matmul(ap_t[:, j * 512:(j + 1) * 512],
                                         z_sb[:, t * 128:(t + 1) * 128],
                                         uc_sb[:, 512 + j * 512:512 + (j + 1) * 512],
                                         start=True, stop=True)
                    aer_sb = opool.tile([128, N], f16, tag="out", name="aer_sb")
                    res_sb = opool.tile([128, N], f16, tag="out", name="res_sb")
                    prn_sb = opool.tile([128, N], f8, tag="out", name="prn_sb")
                    nc.scalar.activation(res_sb[:], ap_t[:], AF.Sigmoid,
                                         bias=sig_bias[:],
                                         scale=RESIST_STEEPNESS)
                    if t % 2 == 0:
                        vcopy(aer_sb[:], ap_t[:])
                    else:
                        nc.scalar.copy(aer_sb[:], ap_t[:])
                    nc.vector.tensor_scalar(prn_sb[:], ap_t[:],
                                            RESIST_THRESHOLD, None,
                                            op0=ALU.is_gt)
                    nc.sync.dma_start(aer_d[t * 128:(t + 1) * 128, :], aer_sb[:])
                    nc.scalar.dma_start(res_d[t * 128:(t + 1) * 128, :], res_sb[:])
                    nc.gpsimd.dma_start(prn_d[t * 128:(t + 1) * 128, :], prn_sb[:])

    nc.compile()
    return nc


_CACHE = {}


def _get_program():
    if "nc" not in _CACHE:
        _CACHE["nc"] = _build_program()
    return _CACHE["nc"]


def _prep_inputs(mask, kernels, scales):
    import ml_dtypes
    bf = ml_dtypes.bfloat16
    f8 = ml_dtypes.float8_e4m3fn

    atp, ctr, cti, U = _host_matrices()

    kers = kernels.astype(np.complex128) * np.sqrt(
        scales.astype(np.float64))[:, None, None]
    # ktR/ktI indexed [v, (k,u)] = transposed per-kernel matrices
    ktR = kers.real.transpose(2, 0, 1).reshape(HK, K * HK).astype(bf)
    ktI = kers.imag.transpose(2, 0, 1).reshape(HK, K * HK).astype(bf)
    cc3 = np.concatenate([ctr, -cti, cti], axis=1).astype(bf)    # [35, 210]
    ctl = np.zeros((99, 2 * NC), np.float32)
    ctl[0:HK, 0:NC] = ctr
    ctl[64:99, 0:NC] = -cti
    ctl[0:HK, NC:2 * NC] = cti
    ctl[64:99, NC:2 * NC] = ctr
    ctl = ctl.astype(bf)                                          # [99, 140]
    ut = U.T                                                      # [70, 1024]
    uc = [np.concatenate([U[h * 512:(h + 1) * 512, :].T, ut],
                         axis=1).astype(np.float16) for h in range(2)]
    atp_f32 = atp.astype(np.float32)
    mask_f8 = np.asarray(mask, np.float32).astype(f8)
    return mask_f8, atp_f32.astype(f8), atp_f32.astype(bf), ktR, ktI, cc3, ctl, uc


# ---------------------------------------------------------------- entry point
def kernel(mask, kernels, kernels_ct, scales):
    """Full inputs in, full outputs out.  Shards over 8 NeuronCores internally."""
    from concourse.bass_utils import run_bass_kernel_spmd

    kernels = np.asarray(kernels, np.complex64)
    scales = np.asarray(scales, np.float32)
    mask_f8, atp8, atp_bf, ktR, ktI, cc3, ctl, uc = _prep_inputs(
        mask, kernels, scales)

    nc = _get_program()
    in_maps = []
    for c in range(8):
        b, h = c // 2, c % 2
        in_maps.append({
            "x": mask_f8[b],
            "atp8": atp8,
            "atp": atp_bf,
            "ktr": ktR,
            "kti": ktI,
            "cc3": cc3,
            "ctl": ctl,
            "uc": uc[h],
        })

    trace = bool(int(os.environ.get("BASS_KERNEL_TRACE", "0")))
    res = run_bass_kernel_spmd(nc, in_maps, core_ids=list(range(8)), trace=trace)
    _CACHE["last_results"] = res

    aerial = np.empty((B, N, N), np.float32)
    resist = np.empty((B, N, N), np.float32)
    printed = np.empty((B, N, N), np.float32)
    for c in range(8):
        b, h = c // 2, c % 2
        rows = slice(h * 512, (h + 1) * 512)
        aerial[b, rows, :] = res.results[c]["aer16"].astype(np.float32)
        resist[b, rows, :] = res.results[c]["res16"].astype(np.float32)
        printed[b, rows, :] = np.asarray(
            res.results[c]["prn8"]).astype(np.float32)
    return aerial, resist, printed
